# revision 7
# baseline (speedup 1.0000x reference)
"""BertSelfAttention Trainium2 kernel.

Full inputs in, full output out. Sharding: 8 cores = (batch b in {0,1}) x
(head-group hg in {0..3}); each core computes 4 heads of one batch and
produces the output feature slice out[b, :, hg*256:(hg+1)*256].

Per-core schedule (v2 — ACT-bound software pipeline):
  The exp of the 16.8M scores per core runs on the Scalar (ACT) engine at
  1 elem/cycle/lane @ 1.2 GHz => ~109us floor + per-instruction overhead.
  ACT is therefore the binding engine; the design minimizes ACT
  instructions (80 exps of 1-2k cols via a 4-bank + 3-bank PSUM
  ping-pong) and keeps the PE (123.5us of fp16 matmul cols at 2.4 GHz)
  saturated underneath it:

  - scores are produced side-sequentially per iteration (it = (hp, qb)),
    in k-tile batches [4,3,4,3,2] alternating the two scores PSUM pools;
  - each batch is exp'd in ONE ACTIVATE (bias=-4, scale=1/8) into a
    persistent per-side e-buffer [128, 8192] f16;
  - ctx consumes e per-batch (lag 2) into a shared 1-bank PSUM work tile,
    accumulated across batches in SBUF f32 by the DVE, so the kernel has
    no big serial exp->ctx tail;
  - Q/K/V projections are filler work, scheduled by DMA arrival to keep
    the PE dense from ~7us (HAM clock stays at 8/8 = 2.4 GHz);
  - x is DMA'd as two [8][128][1024] tensors (2KB lines) so the whole
    input load finishes ~18us while projections overlap it.
"""

import numpy as np

B = 2
S = 2048
H = 1024
NH = 16
HD = 64

NCORES = 8
HPC = 4          # heads per core
DS = HPC * HD    # 256 output dims per core
FT = H // 128    # 8 f-tiles (contraction tiles for projections)
KT = S // 128    # 16 key tiles
QB = 4           # q blocks of 512
QBS = 512
VW = HPC * (HD + 1)  # 260: 4 heads x (64 V dims + 1 em column)

EXP_BIAS = -4.0  # uniform shift inside exp; cancels in softmax, guards fp16

# scores k-tile batches per side: (k0, nk, pool) — pools alternate A(4bk)/B(2bk)
BATCH_PAT = [(0, 4, 0), (4, 2, 1), (6, 4, 0), (10, 2, 1), (12, 4, 0)]
NB_BATCH = len(BATCH_PAT)

_CACHE = {}


def _build_program(split_waits=True):
    import concourse.bass as bass
    import concourse.mybir as mybir
    import concourse.tile as tile
    from concourse.vector_clock import ScopedClock

    f32 = mybir.dt.float32
    f16 = mybir.dt.float16
    AF = mybir.ActivationFunctionType
    OP = mybir.AluOpType

    class SplitDrainTileContext(tile.TileContext):
        """The walrus build here rejects instructions with more than one
        sync wait ("Too many sync wait commands"); hoist excess waits onto
        preceding same-engine NOPs."""

        MAX_WAITS_PER_DRAIN = 1
        split_waits_enabled = True

        def _drain_and_barrier(self, tick_clock, wait_clock):
            drain_inst = self.nc.sync.drain()
            wait_clock.add_sem_waits(
                drain_inst.ins, ScopedClock({None: tick_clock.global_clock})
            )
            self.nc.all_engine_barrier()
            assert self.sems is not None
            popped = self.nc._tile_sem_poison_stack.pop()
            assert popped is self._sem_poison
            self.nc.clear_and_free_semaphores(list(self.sems.allocated().values()))
            self.nc.all_engine_barrier()
            if self.split_waits_enabled:
                self._split_multi_waits()

        def _split_multi_waits(self):
            k = self.MAX_WAITS_PER_DRAIN
            nc = self.nc
            for bb in nc.bb_map.values():
                il = bb.bb.instructions
                new = []
                for inst in il:
                    si = getattr(inst, "sync_info", None)
                    waits = list(si.on_wait) if si is not None and si.on_wait else []
                    if len(waits) > k:
                        for j in range(0, len(waits) - k, k):
                            nop = mybir.InstNoOp(
                                name=nc.get_next_instruction_name(),
                                engine=inst.engine,
                                sync_info=mybir.SyncInfo(
                                    on_wait=waits[j : j + k], on_update=[]
                                ),
                                bass_nofuse=True,
                            )
                            new.append(nop)
                        inst.sync_info = mybir.SyncInfo(
                            on_wait=waits[len(waits) - k :],
                            on_update=list(si.on_update) if si.on_update else [],
                        )
                    new.append(inst)
                il[:] = new

    nc = bass.Bass("TRN2", target_bir_lowering=False, debug=False,
                   num_devices=NCORES)

    # DRAM inputs (per-core layouts prepared host-side)
    wkA_d = nc.dram_tensor("wkA", [128, FT * 256], f16, kind="ExternalInput")
    wqA_d = nc.dram_tensor("wqA", [128, FT * 256], f16, kind="ExternalInput")
    wvA_d = nc.dram_tensor("wvA", [128, FT * VW], f16, kind="ExternalInput")
    xn01_d = nc.dram_tensor("xn01", [FT, 128, 1024], f16, kind="ExternalInput")
    xn23_d = nc.dram_tensor("xn23", [FT, 128, 1024], f16, kind="ExternalInput")
    bq_d = nc.dram_tensor("bq", [2, 128, 1], f32, kind="ExternalInput")
    bk_d = nc.dram_tensor("bk", [2, 128, 1], f32, kind="ExternalInput")
    bvb_d = nc.dram_tensor("bvb", [128, DS], f32, kind="ExternalInput")
    em_d = nc.dram_tensor("em", [128, KT], f32, kind="ExternalInput")
    out_d = nc.dram_tensor("out", [S, DS], f32, kind="ExternalOutput")

    SplitDrainTileContext.split_waits_enabled = split_waits
    with SplitDrainTileContext(nc) as tc:
        from contextlib import ExitStack

        with ExitStack() as ctx:
            const = ctx.enter_context(tc.tile_pool(name="const", bufs=1))
            wpool = ctx.enter_context(tc.tile_pool(name="wpool", bufs=1))
            xpool = ctx.enter_context(tc.tile_pool(name="xpool", bufs=1))
            qk = ctx.enter_context(tc.tile_pool(name="qk", bufs=1))
            vp = ctx.enter_context(tc.tile_pool(name="vp", bufs=1))
            epool = ctx.enter_context(tc.tile_pool(name="epool", bufs=1))
            apool = ctx.enter_context(tc.tile_pool(name="apool", bufs=1))
            opool = ctx.enter_context(tc.tile_pool(name="opool", bufs=1))
            rpool = ctx.enter_context(tc.tile_pool(name="rpool", bufs=1))

            # ---- constants / small DMAs ----
            bq_sb = [const.tile([128, 1], f32, tag=f"bq{m}", bufs=1,
                                name=f"bq_sb{m}") for m in range(2)]
            bk_sb = [const.tile([128, 1], f32, tag=f"bk{m}", bufs=1,
                                name=f"bk_sb{m}") for m in range(2)]
            bvb_sb = const.tile([128, DS], f32, tag="bvb", bufs=1, name="bvb_sb")
            em_sb = const.tile([128, KT], f32, tag="em", bufs=1, name="em_sb")
            ebias = const.tile([128, 1], f32, tag="ebias", bufs=1, name="ebias")

            # ---- big persistent SBUF ----
            wk_sb = wpool.tile([128, FT * 256], f16, tag="wk", bufs=1, name="wk")
            wq_sb = wpool.tile([128, FT * 256], f16, tag="wq", bufs=1, name="wq")
            wv_sb = wpool.tile([128, FT * VW], f16, tag="wv", bufs=1, name="wv")
            x01 = [xpool.tile([128, 1024], f16, tag=f"x01_{ft}", bufs=1,
                              name=f"x01_{ft}") for ft in range(FT)]
            x23 = [xpool.tile([128, 1024], f16, tag=f"x23_{ft}", bufs=1,
                              name=f"x23_{ft}") for ft in range(FT)]

            def xt(ft, nb):
                src = x01[ft] if nb < 2 else x23[ft]
                o = (nb % 2) * 512
                return src[:, o:o + 512]

            qt = [qk.tile([128, S], f16, tag=f"qt{m}", bufs=1, name=f"qt{m}")
                  for m in range(2)]
            kt_sb = [qk.tile([128, S], f16, tag=f"kt{m}", bufs=1, name=f"kt{m}")
                     for m in range(2)]
            vones = [vp.tile([128, VW], f16, tag=f"v{st}", bufs=1,
                             name=f"vones{st}") for st in range(KT)]
            # persistent per-side exp buffers, one iteration at a time
            esb = [epool.tile([128, KT * QBS], f16, tag=f"e{s}", bufs=1,
                              name=f"e{s}") for s in range(2)]
            # per-side ctx accumulators (f32): 4 qq x (64 ctx + 1 sum)
            acc = [apool.tile([128, 4 * (HD + 1)], f32, tag=f"acc{s}", bufs=1,
                              name=f"acc{s}") for s in range(2)]

            # ---- input DMAs in arrival-priority order ----
            nc.sync.dma_start(wk_sb[:], wkA_d.ap())
            for ft in range(FT):
                nc.sync.dma_start(x01[ft][:], xn01_d.ap()[ft])
            nc.sync.dma_start(wq_sb[:], wqA_d.ap())
            for m in range(2):
                nc.sync.dma_start(bq_sb[m][:], bq_d.ap()[m])
                nc.sync.dma_start(bk_sb[m][:], bk_d.ap()[m])
            nc.sync.dma_start(wv_sb[:], wvA_d.ap())
            nc.sync.dma_start(bvb_sb[:], bvb_d.ap())
            nc.sync.dma_start(em_sb[:], em_d.ap())
            for ft in range(FT):
                nc.sync.dma_start(x23[ft][:], xn23_d.ap()[ft])

            nc.vector.memset(ebias[:], EXP_BIAS)
            # warm the ACT exp table while DMAs run
            warm = const.tile([128, 1], f32, tag="warm", bufs=1, name="warm")
            nc.scalar.activation(warm[:], ebias[:], AF.Exp)

            # ---- PSUM pools: scores A (4 banks) + B (2) + work (2x1) ----
            ps_a = ctx.enter_context(
                tc.tile_pool(name="ps_a", bufs=1, space="PSUM"))
            ps_b = ctx.enter_context(
                tc.tile_pool(name="ps_b", bufs=1, space="PSUM"))
            ps_w = ctx.enter_context(
                tc.tile_pool(name="ps_w", bufs=2, space="PSUM"))

            mm = nc.tensor.matmul

            # ---- work units ----
            def qk_proj_block(w_sb, bias_sb, dst, m, nb):
                ns = slice(nb * QBS, (nb + 1) * QBS)
                ps = ps_w.tile([128, QBS], f32, tag="w", name="pspj")
                for ft in range(FT):
                    mm(ps[:],
                       w_sb[:, ft * 256 + m * 128: ft * 256 + (m + 1) * 128],
                       xt(ft, nb),
                       start=(ft == 0), stop=(ft == FT - 1))
                nc.vector.tensor_scalar_add(dst[:, ns], ps[:], bias_sb[:])

            def v_proj_block(st):
                nb, within = divmod(st, 4)
                ws = slice(within * 128, (within + 1) * 128)
                ps = ps_w.tile([128, QBS], f32, tag="w", name="pspjv")
                for ft in range(FT):
                    mm(ps[:, 0:VW],
                       xt(ft, nb)[:, ws],
                       wv_sb[:, ft * VW:(ft + 1) * VW],
                       start=(ft == 0), stop=(ft == FT - 1))
                nc.vector.tensor_scalar_mul(
                    vones[st][:], ps[:, 0:VW], em_sb[:, st:st + 1])
                for hh in range(HPC):
                    c = hh * (HD + 1) + HD
                    nc.vector.tensor_copy(
                        vones[st][:, c:c + 1], em_sb[:, st:st + 1])

            def scores_batch(it, side, bi):
                hp, qb = divmod(it, QB)
                k0, nk, pool = BATCH_PAT[bi]
                p0 = side * 64
                qs = slice(qb * QBS, (qb + 1) * QBS)
                pl = ps_a if pool == 0 else ps_b
                w = nk * QBS
                ps = pl.tile([128, 4 * QBS if pool == 0 else 2 * QBS], f32,
                             tag="sc", name=f"ps{'AB'[pool]}")
                for j in range(nk):
                    ktile = k0 + j
                    ks = slice(ktile * 128, (ktile + 1) * 128)
                    js = slice(j * QBS, (j + 1) * QBS)
                    mm(ps[:, js],
                       kt_sb[hp][p0:p0 + 64, ks], qt[hp][p0:p0 + 64, qs],
                       tile_position=(p0, 0))
                es = slice(k0 * QBS, k0 * QBS + w)
                nc.scalar.activation(esb[side][:, es], ps[:, 0:w],
                                     AF.Exp, bias=ebias[:], scale=0.125)

            def ctx_batch(it, side, bi):
                hp, _ = divmod(it, QB)
                hh = 2 * hp + side
                k0, nk, _ = BATCH_PAT[bi]
                e = esb[side]
                ps = ps_w.tile([128, QBS], f32, tag="w", name="pscx")
                for qq in range(4):
                    dst = ps[:, qq * (HD + 1):(qq + 1) * (HD + 1)]
                    for j in range(nk):
                        ktile = k0 + j
                        lo = ktile * QBS + qq * 128
                        mm(dst,
                           e[:, lo:lo + 128],
                           vones[ktile][:, hh * (HD + 1):(hh + 1) * (HD + 1)],
                           start=(j == 0), stop=(j == nk - 1))
                if bi == 0:
                    nc.vector.tensor_copy(acc[side][:], ps[:, 0:4 * (HD + 1)])
                else:
                    nc.vector.tensor_add(acc[side][:], acc[side][:],
                                         ps[:, 0:4 * (HD + 1)])

            def ctx_final(it, side, ots):
                hp, qb = divmod(it, QB)
                hh = 2 * hp + side
                for qq in range(4):
                    if side == 0:
                        ot = opool.tile([128, 128], f32, tag="ot", bufs=8,
                                        name="ot")
                        ots.append(ot)
                    else:
                        ot = ots[qq]
                    r = rpool.tile([128, 1], f32, tag="r", bufs=8, name="r")
                    a = acc[side]
                    nc.vector.reciprocal(
                        r[:], a[:, qq * (HD + 1) + HD: qq * (HD + 1) + HD + 1])
                    nc.vector.scalar_tensor_tensor(
                        ot[:, side * 64:(side + 1) * 64],
                        a[:, qq * (HD + 1): qq * (HD + 1) + HD], r[:],
                        bvb_sb[:, hh * HD:(hh + 1) * HD],
                        op0=OP.mult, op1=OP.add)
                    if side == 1:
                        qt_idx = qb * 4 + qq
                        nc.sync.dma_start(
                            out_d.ap()[qt_idx * 128:(qt_idx + 1) * 128,
                                       hp * 128:(hp + 1) * 128],
                            ot[:])

            # ---- emission schedule ----
            # Slot atoms per iteration (side-sequential scores batches;
            # ctx lags its exp by 2 batches; each side's last 2 ctx
            # batches + final carry into the next iteration's start):
            #  0:(s,A,0) 1:(s,A,1) 2:(s,A,2) 3:(c,A,0) 4:(s,A,3) 5:(c,A,1)
            #  6:(s,A,4) 7:(c,A,2) 8:(s,B,0) 9:(s,B,1) 10:(s,B,2) 11:(c,B,0)
            #  12:(s,B,3) 13:(c,B,1) 14:(s,B,4) 15:(c,B,2)
            def K0(nb):
                return lambda: qk_proj_block(wk_sb, bk_sb[0], kt_sb[0], 0, nb)

            def K1(nb):
                return lambda: qk_proj_block(wk_sb, bk_sb[1], kt_sb[1], 1, nb)

            def Q0(nb):
                return lambda: qk_proj_block(wq_sb, bq_sb[0], qt[0], 0, nb)

            def Q1(nb):
                return lambda: qk_proj_block(wq_sb, bq_sb[1], qt[1], 1, nb)

            def V(st):
                return lambda: v_proj_block(st)

            # fillers[it] = list of (atom_idx, thunk): emit before that atom.
            # Placement honors DMA arrival order (wk, x01, wq, wv, x23) and
            # consumer deadlines (scores need kt/qt; ctx batch bi reads
            # vones[k0..k0+nk-1]; it N+1 reads carry ctx of it N).
            fillers = {
                0: [(1, K0(1)), (2, K0(2)),
                    (3, V(0)), (3, V(1)), (3, V(2)), (3, V(3)),
                    (4, K0(3)),
                    (5, V(4)), (5, V(5)),
                    (7, V(6)), (7, V(7)), (7, V(8)), (7, V(9)),
                    (8, V(10)), (9, V(11)), (10, V(12)),
                    (12, V(13)), (13, V(14)), (13, V(15)),
                    (14, Q0(1))],
                1: [(0, Q0(2)), (4, K1(0)), (8, K1(1))],
                2: [(0, Q0(3)), (4, K1(2)), (8, K1(3))],
                3: [(0, Q1(0)), (8, Q1(1))],
                4: [(0, Q1(2))],
                5: [(0, Q1(3))],
            }

            # head: minimal work to unblock it0's first scores batch
            qk_proj_block(wk_sb, bk_sb[0], kt_sb[0], 0, 0)
            qk_proj_block(wq_sb, bq_sb[0], qt[0], 0, 0)

            slots = []
            for side in range(2):
                for bi in range(NB_BATCH):
                    slots.append(("s", side, bi))
                    if bi >= 2:
                        slots.append(("c", side, bi - 2))

            carry = []  # thunks emitted interleaved at next it's start
            for it in range(8):
                flist = list(fillers.get(it, []))
                cq = list(carry)
                for idx, (kind, side, bi) in enumerate(slots):
                    if cq:
                        cq.pop(0)()
                    while flist and flist[0][0] <= idx:
                        flist.pop(0)[1]()
                    if kind == "s":
                        scores_batch(it, side, bi)
                    else:
                        ctx_batch(it, side, bi)
                for f in flist:
                    f[1]()
                for c in cq:
                    c()
                ots = []
                carry = [
                    (lambda it=it: ctx_batch(it, 0, NB_BATCH - 2)),
                    (lambda it=it: ctx_batch(it, 0, NB_BATCH - 1)),
                    (lambda it=it, ots=ots: ctx_final(it, 0, ots)),
                    (lambda it=it: ctx_batch(it, 1, NB_BATCH - 2)),
                    (lambda it=it: ctx_batch(it, 1, NB_BATCH - 1)),
                    (lambda it=it, ots=ots: ctx_final(it, 1, ots)),
                ]
            for c in carry:
                c()

    return nc


def _get_program(split_waits=True):
    key = ("nc", split_waits)
    if key not in _CACHE:
        _CACHE[key] = _build_program(split_waits)
    return _CACHE[key]


def _make_in_maps(hidden_states, attention_mask, Wq, bq, Wk, bk, Wv, bv):
    hidden = np.ascontiguousarray(np.asarray(hidden_states, dtype=np.float32))
    mask = np.asarray(attention_mask, dtype=np.float32)
    Wq = np.asarray(Wq, dtype=np.float32)
    Wk = np.asarray(Wk, dtype=np.float32)
    Wv = np.asarray(Wv, dtype=np.float32)
    bq = np.asarray(bq, dtype=np.float32)
    bk = np.asarray(bk, dtype=np.float32)
    bv = np.asarray(bv, dtype=np.float32)

    WqT = Wq.T  # [in, out]
    WkT = Wk.T
    WvT = Wv.T

    def pack_w(WT, cols):
        # [H, 256] -> [128, 8*256]: per f-tile 256-col blocks
        w = WT[:, cols].astype(np.float16)  # [1024, 256]
        return np.ascontiguousarray(
            w.reshape(FT, 128, 256).transpose(1, 0, 2).reshape(128, FT * 256))

    in_maps = []
    for c in range(NCORES):
        b, hg = divmod(c, HPC)
        cols = slice(hg * DS, (hg + 1) * DS)
        xT = hidden[b].T.astype(np.float16)  # [1024, 2048]
        xn01 = np.ascontiguousarray(
            xT[:, 0:1024].reshape(FT, 128, 1024))
        xn23 = np.ascontiguousarray(
            xT[:, 1024:2048].reshape(FT, 128, 1024))
        wkA = pack_w(WkT, cols)
        wqA = pack_w(WqT, cols)
        wv_base = WvT[:, cols].astype(np.float16)  # [1024, 256]
        wvA = np.zeros((128, FT * VW), np.float16)
        for ft in range(FT):
            blk = wv_base[ft * 128:(ft + 1) * 128]  # [128, 256]
            for hh in range(HPC):
                wvA[:, ft * VW + hh * (HD + 1): ft * VW + hh * (HD + 1) + HD] \
                    = blk[:, hh * HD:(hh + 1) * HD]
        bq_c = np.ascontiguousarray(bq[cols].reshape(2, 128, 1))
        bk_c = np.ascontiguousarray(bk[cols].reshape(2, 128, 1))
        bvb = np.ascontiguousarray(np.tile(bv[cols][None, :], (128, 1)))
        em = np.ascontiguousarray(
            np.exp(mask[b, 0, 0, :]).reshape(KT, 128).T.astype(np.float32))
        in_maps.append({
            "wkA": wkA, "wqA": wqA, "wvA": wvA,
            "xn01": xn01, "xn23": xn23,
            "bq": bq_c, "bk": bk_c, "bvb": bvb, "em": em,
        })
    return in_maps


def _assemble(results):
    out = np.empty((B, S, H), np.float32)
    for c in range(NCORES):
        b, hg = divmod(c, HPC)
        out[b][:, hg * DS:(hg + 1) * DS] = results[c]["out"]
    return out


def _run(in_maps, trace=False):
    from concourse.bass_utils import run_bass_kernel_spmd
    nc = _get_program()
    return run_bass_kernel_spmd(
        nc, in_maps, core_ids=list(range(NCORES)), trace=trace)


def kernel(**inputs):
    in_maps = _make_in_maps(**inputs)
    res = _run(in_maps, trace=False)
    return _assemble(res.results)


# revision 14
# speedup vs baseline: 1.1787x; 1.1787x over previous
"""BertSelfAttention Trainium2 kernel.

Full inputs in, full output out. Sharding: 8 cores = (batch b in {0,1}) x
(head-group hg in {0..3}); each core computes 4 heads of one batch and
produces the output feature slice out[b, :, hg*256:(hg+1)*256].

Per-core schedule (v2 — ACT-bound software pipeline):
  The exp of the 16.8M scores per core runs on the Scalar (ACT) engine at
  1 elem/cycle/lane @ 1.2 GHz => ~109us floor + per-instruction overhead.
  ACT is therefore the binding engine; the design minimizes ACT
  instructions (80 exps of 1-2k cols via a 4-bank + 3-bank PSUM
  ping-pong) and keeps the PE (123.5us of fp16 matmul cols at 2.4 GHz)
  saturated underneath it:

  - scores are produced side-sequentially per iteration (it = (hp, qb)),
    in k-tile batches [4,3,4,3,2] alternating the two scores PSUM pools;
  - each batch is exp'd in ONE ACTIVATE (bias=-4, scale=1/8) into a
    persistent per-side e-buffer [128, 8192] f16;
  - ctx consumes e per-batch (lag 2) into a shared 1-bank PSUM work tile,
    accumulated across batches in SBUF f32 by the DVE, so the kernel has
    no big serial exp->ctx tail;
  - Q/K/V projections are filler work, scheduled by DMA arrival to keep
    the PE dense from ~7us (HAM clock stays at 8/8 = 2.4 GHz);
  - x is DMA'd as two [8][128][1024] tensors (2KB lines) so the whole
    input load finishes ~18us while projections overlap it.
"""

import numpy as np

B = 2
S = 2048
H = 1024
NH = 16
HD = 64

NCORES = 8
HPC = 4          # heads per core
DS = HPC * HD    # 256 output dims per core
FT = H // 128    # 8 f-tiles (contraction tiles for projections)
KT = S // 128    # 16 key tiles
QB = 4           # q blocks of 512
QBS = 512
VW = HPC * (HD + 1)  # 260: 4 heads x (64 V dims + 1 em column)

EXP_BIAS = -4.0  # uniform shift inside exp; cancels in softmax, guards fp16

# scores k-tile batches per (side, bi): (k0, nk, pool); pool A = 4 banks,
# pool B = 2. Orders chosen so consecutive allocations of the same pool
# always have >=2 batches of other work between them (the WAR wait on the
# previous batch's exp is the main PE stall point).
BATCH_PAT = [
    [(0, 4, 0), (4, 2, 1), (6, 4, 0), (10, 2, 1), (12, 4, 0)],
    [(0, 2, 1), (2, 4, 0), (6, 2, 1), (8, 4, 0), (12, 4, 0)],
]
NB_BATCH = 5

_CACHE = {}


def _build_program(split_waits=True):
    import concourse.bass as bass
    import concourse.mybir as mybir
    import concourse.tile as tile
    from concourse.vector_clock import ScopedClock

    f32 = mybir.dt.float32
    f16 = mybir.dt.float16
    AF = mybir.ActivationFunctionType
    OP = mybir.AluOpType

    class SplitDrainTileContext(tile.TileContext):
        """The walrus build here rejects instructions with more than one
        sync wait ("Too many sync wait commands"); hoist excess waits onto
        preceding same-engine NOPs."""

        MAX_WAITS_PER_DRAIN = 1
        split_waits_enabled = True

        def _drain_and_barrier(self, tick_clock, wait_clock):
            drain_inst = self.nc.sync.drain()
            wait_clock.add_sem_waits(
                drain_inst.ins, ScopedClock({None: tick_clock.global_clock})
            )
            self.nc.all_engine_barrier()
            assert self.sems is not None
            popped = self.nc._tile_sem_poison_stack.pop()
            assert popped is self._sem_poison
            self.nc.clear_and_free_semaphores(list(self.sems.allocated().values()))
            self.nc.all_engine_barrier()
            if self.split_waits_enabled:
                self._split_multi_waits()

        def _split_multi_waits(self):
            k = self.MAX_WAITS_PER_DRAIN
            nc = self.nc
            for bb in nc.bb_map.values():
                il = bb.bb.instructions
                new = []
                for inst in il:
                    si = getattr(inst, "sync_info", None)
                    waits = list(si.on_wait) if si is not None and si.on_wait else []
                    if len(waits) > k:
                        for j in range(0, len(waits) - k, k):
                            nop = mybir.InstNoOp(
                                name=nc.get_next_instruction_name(),
                                engine=inst.engine,
                                sync_info=mybir.SyncInfo(
                                    on_wait=waits[j : j + k], on_update=[]
                                ),
                                bass_nofuse=True,
                            )
                            new.append(nop)
                        inst.sync_info = mybir.SyncInfo(
                            on_wait=waits[len(waits) - k :],
                            on_update=list(si.on_update) if si.on_update else [],
                        )
                    new.append(inst)
                il[:] = new

    nc = bass.Bass("TRN2", target_bir_lowering=False, debug=False,
                   num_devices=NCORES)

    # DRAM inputs (per-core layouts prepared host-side)
    wkA_d = nc.dram_tensor("wkA", [128, FT * 256], f16, kind="ExternalInput")
    wqA_d = nc.dram_tensor("wqA", [128, FT * 256], f16, kind="ExternalInput")
    wvA_d = nc.dram_tensor("wvA", [128, FT * VW], f16, kind="ExternalInput")
    xn01_d = nc.dram_tensor("xn01", [FT, 128, 1024], f16, kind="ExternalInput")
    xn23_d = nc.dram_tensor("xn23", [FT, 128, 1024], f16, kind="ExternalInput")
    bq_d = nc.dram_tensor("bq", [2, 128, 1], f32, kind="ExternalInput")
    bk_d = nc.dram_tensor("bk", [2, 128, 1], f32, kind="ExternalInput")
    bvb_d = nc.dram_tensor("bvb", [128, DS], f32, kind="ExternalInput")
    em_d = nc.dram_tensor("em", [128, KT], f32, kind="ExternalInput")
    out_d = nc.dram_tensor("out", [S, DS], f32, kind="ExternalOutput")

    SplitDrainTileContext.split_waits_enabled = split_waits
    with SplitDrainTileContext(nc) as tc:
        from contextlib import ExitStack

        with ExitStack() as ctx:
            const = ctx.enter_context(tc.tile_pool(name="const", bufs=1))
            wpool = ctx.enter_context(tc.tile_pool(name="wpool", bufs=1))
            xpool = ctx.enter_context(tc.tile_pool(name="xpool", bufs=1))
            qk = ctx.enter_context(tc.tile_pool(name="qk", bufs=1))
            vp = ctx.enter_context(tc.tile_pool(name="vp", bufs=1))
            epool = ctx.enter_context(tc.tile_pool(name="epool", bufs=1))
            apool = ctx.enter_context(tc.tile_pool(name="apool", bufs=1))
            opool = ctx.enter_context(tc.tile_pool(name="opool", bufs=1))
            rpool = ctx.enter_context(tc.tile_pool(name="rpool", bufs=1))

            # ---- constants / small DMAs ----
            bq_sb = [const.tile([128, 1], f32, tag=f"bq{m}", bufs=1,
                                name=f"bq_sb{m}") for m in range(2)]
            bk_sb = [const.tile([128, 1], f32, tag=f"bk{m}", bufs=1,
                                name=f"bk_sb{m}") for m in range(2)]
            bvb_sb = const.tile([128, DS], f32, tag="bvb", bufs=1, name="bvb_sb")
            em_sb = const.tile([128, KT], f32, tag="em", bufs=1, name="em_sb")
            ebias = const.tile([128, 1], f32, tag="ebias", bufs=1, name="ebias")

            # ---- big persistent SBUF ----
            wk_sb = wpool.tile([128, FT * 256], f16, tag="wk", bufs=1, name="wk")
            wq_sb = wpool.tile([128, FT * 256], f16, tag="wq", bufs=1, name="wq")
            wv_sb = wpool.tile([128, FT * VW], f16, tag="wv", bufs=1, name="wv")
            x01 = [xpool.tile([128, 1024], f16, tag=f"x01_{ft}", bufs=1,
                              name=f"x01_{ft}") for ft in range(FT)]
            x23 = [xpool.tile([128, 1024], f16, tag=f"x23_{ft}", bufs=1,
                              name=f"x23_{ft}") for ft in range(FT)]

            def xt(ft, nb):
                src = x01[ft] if nb < 2 else x23[ft]
                o = (nb % 2) * 512
                return src[:, o:o + 512]

            qt = [qk.tile([128, S], f16, tag=f"qt{m}", bufs=1, name=f"qt{m}")
                  for m in range(2)]
            kt_sb = [qk.tile([128, S], f16, tag=f"kt{m}", bufs=1, name=f"kt{m}")
                     for m in range(2)]
            vones = [vp.tile([128, VW], f16, tag=f"v{st}", bufs=1,
                             name=f"vones{st}") for st in range(KT)]
            # persistent per-side exp buffers, one iteration at a time
            esb = [epool.tile([128, KT * QBS], f16, tag=f"e{s}", bufs=1,
                              name=f"e{s}") for s in range(2)]
            # per-side ctx accumulators (f32): 4 qq x (64 ctx + 1 sum)
            acc = [apool.tile([128, 4 * (HD + 1)], f32, tag=f"acc{s}", bufs=1,
                              name=f"acc{s}") for s in range(2)]

            # ---- input DMAs in arrival-priority order ----
            for m in range(2):
                nc.sync.dma_start(bq_sb[m][:], bq_d.ap()[m])
                nc.sync.dma_start(bk_sb[m][:], bk_d.ap()[m])
            nc.sync.dma_start(em_sb[:], em_d.ap())
            nc.sync.dma_start(wk_sb[:], wkA_d.ap())
            nc.sync.dma_start(wq_sb[:], wqA_d.ap())
            for ft in range(FT):
                nc.sync.dma_start(x01[ft][:], xn01_d.ap()[ft])
            nc.sync.dma_start(wv_sb[:], wvA_d.ap())
            nc.sync.dma_start(bvb_sb[:], bvb_d.ap())
            for ft in range(FT):
                nc.sync.dma_start(x23[ft][:], xn23_d.ap()[ft])

            nc.vector.memset(ebias[:], EXP_BIAS)
            # warm the ACT exp table while DMAs run
            warm = const.tile([128, 1], f32, tag="warm", bufs=1, name="warm")
            nc.scalar.activation(warm[:], ebias[:], AF.Exp)

            # ---- PSUM pools: scores A (4 banks) + B (2) + work (2x1) ----
            ps_a = ctx.enter_context(
                tc.tile_pool(name="ps_a", bufs=1, space="PSUM"))
            ps_b = ctx.enter_context(
                tc.tile_pool(name="ps_b", bufs=1, space="PSUM"))
            ps_w = ctx.enter_context(
                tc.tile_pool(name="ps_w", bufs=2, space="PSUM"))

            mm = nc.tensor.matmul

            # ---- work units ----
            def qk_proj_block(w_sb, bias_sb, dst, m, nb):
                ns = slice(nb * QBS, (nb + 1) * QBS)
                ps = ps_w.tile([128, QBS], f32, tag="w", name="pspj")
                for ft in range(FT):
                    mm(ps[:],
                       w_sb[:, ft * 256 + m * 128: ft * 256 + (m + 1) * 128],
                       xt(ft, nb),
                       start=(ft == 0), stop=(ft == FT - 1))
                nc.vector.tensor_scalar_add(dst[:, ns], ps[:], bias_sb[:])

            def v_proj_block(st):
                nb, within = divmod(st, 4)
                ws = slice(within * 128, (within + 1) * 128)
                ps = ps_w.tile([128, QBS], f32, tag="w", name="pspjv")
                for ft in range(FT):
                    mm(ps[:, 0:VW],
                       xt(ft, nb)[:, ws],
                       wv_sb[:, ft * VW:(ft + 1) * VW],
                       start=(ft == 0), stop=(ft == FT - 1))
                nc.vector.tensor_scalar_mul(
                    vones[st][:], ps[:, 0:VW], em_sb[:, st:st + 1])
                for hh in range(HPC):
                    c = hh * (HD + 1) + HD
                    nc.vector.tensor_copy(
                        vones[st][:, c:c + 1], em_sb[:, st:st + 1])

            def scores_batch(it, side, bi):
                hp, qb = divmod(it, QB)
                k0, nk, pool = BATCH_PAT[side][bi]
                p0 = side * 64
                qs = slice(qb * QBS, (qb + 1) * QBS)
                pl = ps_a if pool == 0 else ps_b
                w = nk * QBS
                ps = pl.tile([128, 4 * QBS if pool == 0 else 2 * QBS], f32,
                             tag="sc", name=f"ps{'AB'[pool]}")
                for j in range(nk):
                    ktile = k0 + j
                    ks = slice(ktile * 128, (ktile + 1) * 128)
                    js = slice(j * QBS, (j + 1) * QBS)
                    mm(ps[:, js],
                       kt_sb[hp][p0:p0 + 64, ks], qt[hp][p0:p0 + 64, qs],
                       tile_position=(p0, 0))
                es = slice(k0 * QBS, k0 * QBS + w)
                nc.scalar.activation(esb[side][:, es], ps[:, 0:w],
                                     AF.Exp, bias=ebias[:], scale=0.125)

            def ctx_batch(it, side, bi):
                hp, _ = divmod(it, QB)
                hh = 2 * hp + side
                k0, nk, _ = BATCH_PAT[side][bi]
                e = esb[side]
                ps = ps_w.tile([128, QBS], f32, tag="w", name="pscx")
                for qq in range(4):
                    dst = ps[:, qq * (HD + 1):(qq + 1) * (HD + 1)]
                    for j in range(nk):
                        ktile = k0 + j
                        lo = ktile * QBS + qq * 128
                        mm(dst,
                           e[:, lo:lo + 128],
                           vones[ktile][:, hh * (HD + 1):(hh + 1) * (HD + 1)],
                           start=(j == 0), stop=(j == nk - 1))
                if bi == 0:
                    nc.vector.tensor_copy(acc[side][:], ps[:, 0:4 * (HD + 1)])
                else:
                    nc.vector.tensor_add(acc[side][:], acc[side][:],
                                         ps[:, 0:4 * (HD + 1)])

            def ctx_final(it, side, ots):
                hp, qb = divmod(it, QB)
                hh = 2 * hp + side
                for qq in range(4):
                    if side == 0:
                        ot = opool.tile([128, 128], f32, tag="ot", bufs=8,
                                        name="ot")
                        ots.append(ot)
                    else:
                        ot = ots[qq]
                    r = rpool.tile([128, 1], f32, tag="r", bufs=8, name="r")
                    a = acc[side]
                    nc.vector.reciprocal(
                        r[:], a[:, qq * (HD + 1) + HD: qq * (HD + 1) + HD + 1])
                    nc.vector.scalar_tensor_tensor(
                        ot[:, side * 64:(side + 1) * 64],
                        a[:, qq * (HD + 1): qq * (HD + 1) + HD], r[:],
                        bvb_sb[:, hh * HD:(hh + 1) * HD],
                        op0=OP.mult, op1=OP.add)
                    if side == 1:
                        qt_idx = qb * 4 + qq
                        nc.sync.dma_start(
                            out_d.ap()[qt_idx * 128:(qt_idx + 1) * 128,
                                       hp * 128:(hp + 1) * 128],
                            ot[:])

            # ---- emission schedule ----
            # Slot atoms per iteration: side A's 5 scores batches, ctx
            # lagging 3 batches; side A's trailing ctx batches spread into
            # side B's scores; side B's trailing ctx + final carry into
            # the next iteration's start (so every PE wait point has
            # independent work queued behind it and no gap grows past
            # ~1us — the HAM clock gate re-throttles on longer idles):
            #  0:(s,A,0) 1:(s,A,1) 2:(s,A,2) 3:(s,A,3) 4:(c,A,0) 5:(s,A,4)
            #  6:(c,A,1) 7:(s,B,0) 8:(c,A,2) 9:(s,B,1) 10:(c,A,3) 11:(s,B,2)
            #  12:(c,A,4) 13:(F,A) 14:(s,B,3) 15:(c,B,0) 16:(s,B,4) 17:(c,B,1)
            def K0(nb):
                return lambda: qk_proj_block(wk_sb, bk_sb[0], kt_sb[0], 0, nb)

            def K1(nb):
                return lambda: qk_proj_block(wk_sb, bk_sb[1], kt_sb[1], 1, nb)

            def Q0(nb):
                return lambda: qk_proj_block(wq_sb, bq_sb[0], qt[0], 0, nb)

            def Q1(nb):
                return lambda: qk_proj_block(wq_sb, bq_sb[1], qt[1], 1, nb)

            def V(st):
                return lambda: v_proj_block(st)

            # fillers[it] = list of (atom_idx, thunk): emit before that atom.
            # Placement honors DMA arrival order (wk, wq, x01, wv, x23) and
            # consumer deadlines (scores need kt/qt; ctx batch bi reads
            # vones[k0..k0+nk-1]; it N+1 reads carry ctx of it N).
            fillers = {
                0: [(1, K0(1)),
                    (2, V(0)), (2, V(1)), (2, V(2)), (2, V(3)),
                    (2, K1(0)), (2, K1(1)), (2, Q1(0)), (2, K0(2)),
                    (3, V(4)), (3, V(5)),
                    (5, K0(3)),
                    (7, V(6)), (7, V(7)), (7, V(8)), (7, V(9)),
                    (9, V(10)), (9, V(11)),
                    (11, V(12)), (11, V(13)), (12, V(14)), (12, V(15)),
                    (17, Q0(1))],
                1: [(0, Q0(2)), (9, K1(2))],
                2: [(0, Q0(3)), (9, K1(3))],
                4: [(0, Q1(1))],
                5: [(0, Q1(2))],
                6: [(0, Q1(3))],
            }

            # head: minimal work to unblock it0's first scores batch
            qk_proj_block(wk_sb, bk_sb[0], kt_sb[0], 0, 0)
            qk_proj_block(wq_sb, bq_sb[0], qt[0], 0, 0)

            atoms = []
            for bi in range(NB_BATCH):
                atoms.append(("s", 0, bi))
                if bi >= 3:
                    atoms.append(("c", 0, bi - 3))
            for bi in range(NB_BATCH):
                atoms.append(("s", 1, bi))
                atoms.append(("c", 0, bi + 2) if bi <= 2 else ("c", 1, bi - 3))
                if bi == 2:
                    atoms.append(("F", 0, 0))

            carry = []  # side B trailing work, emitted at next it's start
            ots_map = {}
            for it in range(8):
                ots_map[it] = []
                flist = list(fillers.get(it, []))
                for c in carry[:2]:
                    c()
                cq = list(carry[2:])
                for idx, atom in enumerate(atoms):
                    while flist and flist[0][0] <= idx:
                        flist.pop(0)[1]()
                    kind, side, bi = atom
                    if kind == "s":
                        scores_batch(it, side, bi)
                    elif kind == "c":
                        ctx_batch(it, side, bi)
                    else:
                        ctx_final(it, 0, ots_map[it])
                    if cq:
                        cq.pop(0)()
                for f in flist:
                    f[1]()
                for c in cq:
                    c()
                carry = [
                    (lambda it=it: ctx_batch(it, 1, 2)),
                    (lambda it=it: ctx_batch(it, 1, 3)),
                    (lambda it=it: ctx_batch(it, 1, 4)),
                    (lambda it=it: ctx_final(it, 1, ots_map[it])),
                ]
            for c in carry:
                c()

    return nc


def _get_program(split_waits=True):
    key = ("nc", split_waits)
    if key not in _CACHE:
        _CACHE[key] = _build_program(split_waits)
    return _CACHE[key]


def _make_in_maps(hidden_states, attention_mask, Wq, bq, Wk, bk, Wv, bv):
    hidden = np.ascontiguousarray(np.asarray(hidden_states, dtype=np.float32))
    mask = np.asarray(attention_mask, dtype=np.float32)
    Wq = np.asarray(Wq, dtype=np.float32)
    Wk = np.asarray(Wk, dtype=np.float32)
    Wv = np.asarray(Wv, dtype=np.float32)
    bq = np.asarray(bq, dtype=np.float32)
    bk = np.asarray(bk, dtype=np.float32)
    bv = np.asarray(bv, dtype=np.float32)

    WqT = Wq.T  # [in, out]
    WkT = Wk.T
    WvT = Wv.T

    def pack_w(WT, cols):
        # [H, 256] -> [128, 8*256]: per f-tile 256-col blocks
        w = WT[:, cols].astype(np.float16)  # [1024, 256]
        return np.ascontiguousarray(
            w.reshape(FT, 128, 256).transpose(1, 0, 2).reshape(128, FT * 256))

    in_maps = []
    for c in range(NCORES):
        b, hg = divmod(c, HPC)
        cols = slice(hg * DS, (hg + 1) * DS)
        xT = hidden[b].T.astype(np.float16)  # [1024, 2048]
        xn01 = np.ascontiguousarray(
            xT[:, 0:1024].reshape(FT, 128, 1024))
        xn23 = np.ascontiguousarray(
            xT[:, 1024:2048].reshape(FT, 128, 1024))
        wkA = pack_w(WkT, cols)
        wqA = pack_w(WqT, cols)
        wv_base = WvT[:, cols].astype(np.float16)  # [1024, 256]
        wvA = np.zeros((128, FT * VW), np.float16)
        for ft in range(FT):
            blk = wv_base[ft * 128:(ft + 1) * 128]  # [128, 256]
            for hh in range(HPC):
                wvA[:, ft * VW + hh * (HD + 1): ft * VW + hh * (HD + 1) + HD] \
                    = blk[:, hh * HD:(hh + 1) * HD]
        bq_c = np.ascontiguousarray(bq[cols].reshape(2, 128, 1))
        bk_c = np.ascontiguousarray(bk[cols].reshape(2, 128, 1))
        bvb = np.ascontiguousarray(np.tile(bv[cols][None, :], (128, 1)))
        em = np.ascontiguousarray(
            np.exp(mask[b, 0, 0, :]).reshape(KT, 128).T.astype(np.float32))
        in_maps.append({
            "wkA": wkA, "wqA": wqA, "wvA": wvA,
            "xn01": xn01, "xn23": xn23,
            "bq": bq_c, "bk": bk_c, "bvb": bvb, "em": em,
        })
    return in_maps


def _assemble(results):
    out = np.empty((B, S, H), np.float32)
    for c in range(NCORES):
        b, hg = divmod(c, HPC)
        out[b][:, hg * DS:(hg + 1) * DS] = results[c]["out"]
    return out


def _run(in_maps, trace=False):
    from concourse.bass_utils import run_bass_kernel_spmd
    nc = _get_program()
    return run_bass_kernel_spmd(
        nc, in_maps, core_ids=list(range(NCORES)), trace=trace)


def kernel(**inputs):
    in_maps = _make_in_maps(**inputs)
    res = _run(in_maps, trace=False)
    return _assemble(res.results)


# revision 18
# speedup vs baseline: 1.2357x; 1.0484x over previous
"""BertSelfAttention Trainium2 kernel.

Full inputs in, full output out. Sharding: 8 cores = (batch b in {0,1}) x
(head-group hg in {0..3}); each core computes 4 heads of one batch and
produces the output feature slice out[b, :, hg*256:(hg+1)*256].

Per-core schedule (v2 — ACT-bound software pipeline):
  The exp of the 16.8M scores per core runs on the Scalar (ACT) engine at
  1 elem/cycle/lane @ 1.2 GHz => ~109us floor + per-instruction overhead.
  ACT is therefore the binding engine; the design minimizes ACT
  instructions (80 exps of 1-2k cols via a 4-bank + 3-bank PSUM
  ping-pong) and keeps the PE (123.5us of fp16 matmul cols at 2.4 GHz)
  saturated underneath it:

  - scores are produced side-sequentially per iteration (it = (hp, qb)),
    in k-tile batches [4,3,4,3,2] alternating the two scores PSUM pools;
  - each batch is exp'd in ONE ACTIVATE (bias=-4, scale=1/8) into a
    persistent per-side e-buffer [128, 8192] f16;
  - ctx consumes e per-batch (lag 2) into a shared 1-bank PSUM work tile,
    accumulated across batches in SBUF f32 by the DVE, so the kernel has
    no big serial exp->ctx tail;
  - Q/K/V projections are filler work, scheduled by DMA arrival to keep
    the PE dense from ~7us (HAM clock stays at 8/8 = 2.4 GHz);
  - x is DMA'd as two [8][128][1024] tensors (2KB lines) so the whole
    input load finishes ~18us while projections overlap it.
"""

import numpy as np

B = 2
S = 2048
H = 1024
NH = 16
HD = 64

NCORES = 8
HPC = 4          # heads per core
DS = HPC * HD    # 256 output dims per core
FT = H // 128    # 8 f-tiles (contraction tiles for projections)
KT = S // 128    # 16 key tiles
QB = 4           # q blocks of 512
QBS = 512
VW = HPC * (HD + 1)  # 260: 4 heads x (64 V dims + 1 em column)

EXP_BIAS = -4.0  # uniform shift inside exp; cancels in softmax, guards fp16

# scores k-tile batches per (side, bi): (k0, nk, pool); pool A = 4 banks,
# pool B = 2. Orders chosen so consecutive allocations of the same pool
# always have >=2 batches of other work between them (the WAR wait on the
# previous batch's exp is the main PE stall point).
BATCH_PAT = [
    [(0, 4, 0), (4, 2, 1), (6, 4, 0), (10, 2, 1), (12, 4, 0)],
    [(0, 2, 1), (2, 4, 0), (6, 2, 1), (8, 4, 0), (12, 4, 0)],
]
NB_BATCH = 5

_CACHE = {}


def _build_program(split_waits=True):
    import concourse.bass as bass
    import concourse.mybir as mybir
    import concourse.tile as tile
    from concourse.tile_rust import add_dep_helper
    from concourse.vector_clock import ScopedClock

    f32 = mybir.dt.float32
    f16 = mybir.dt.float16
    AF = mybir.ActivationFunctionType
    OP = mybir.AluOpType

    class SplitDrainTileContext(tile.TileContext):
        """The walrus build here rejects instructions with more than one
        sync wait ("Too many sync wait commands"); hoist excess waits onto
        preceding same-engine NOPs."""

        MAX_WAITS_PER_DRAIN = 1
        split_waits_enabled = True

        def _drain_and_barrier(self, tick_clock, wait_clock):
            drain_inst = self.nc.sync.drain()
            wait_clock.add_sem_waits(
                drain_inst.ins, ScopedClock({None: tick_clock.global_clock})
            )
            self.nc.all_engine_barrier()
            assert self.sems is not None
            popped = self.nc._tile_sem_poison_stack.pop()
            assert popped is self._sem_poison
            self.nc.clear_and_free_semaphores(list(self.sems.allocated().values()))
            self.nc.all_engine_barrier()
            if self.split_waits_enabled:
                self._split_multi_waits()

        def _split_multi_waits(self):
            k = self.MAX_WAITS_PER_DRAIN
            nc = self.nc
            for bb in nc.bb_map.values():
                il = bb.bb.instructions
                new = []
                for inst in il:
                    si = getattr(inst, "sync_info", None)
                    waits = list(si.on_wait) if si is not None and si.on_wait else []
                    if len(waits) > k:
                        for j in range(0, len(waits) - k, k):
                            nop = mybir.InstNoOp(
                                name=nc.get_next_instruction_name(),
                                engine=inst.engine,
                                sync_info=mybir.SyncInfo(
                                    on_wait=waits[j : j + k], on_update=[]
                                ),
                                bass_nofuse=True,
                            )
                            new.append(nop)
                        inst.sync_info = mybir.SyncInfo(
                            on_wait=waits[len(waits) - k :],
                            on_update=list(si.on_update) if si.on_update else [],
                        )
                    new.append(inst)
                il[:] = new

    nc = bass.Bass("TRN2", target_bir_lowering=False, debug=False,
                   num_devices=NCORES)

    # DRAM inputs (per-core layouts prepared host-side)
    wkA_d = nc.dram_tensor("wkA", [128, FT * 256], f16, kind="ExternalInput")
    wqA_d = nc.dram_tensor("wqA", [128, FT * 256], f16, kind="ExternalInput")
    wvA_d = nc.dram_tensor("wvA", [128, FT * VW], f16, kind="ExternalInput")
    xn01_d = nc.dram_tensor("xn01", [FT, 128, 1024], f16, kind="ExternalInput")
    xn23_d = nc.dram_tensor("xn23", [FT, 128, 1024], f16, kind="ExternalInput")
    bq_d = nc.dram_tensor("bq", [2, 128, 1], f32, kind="ExternalInput")
    bk_d = nc.dram_tensor("bk", [2, 128, 1], f32, kind="ExternalInput")
    bvb_d = nc.dram_tensor("bvb", [128, DS], f32, kind="ExternalInput")
    em_d = nc.dram_tensor("em", [128, KT], f32, kind="ExternalInput")
    out_d = nc.dram_tensor("out", [S, DS], f32, kind="ExternalOutput")

    SplitDrainTileContext.split_waits_enabled = split_waits
    with SplitDrainTileContext(nc) as tc:
        from contextlib import ExitStack

        with ExitStack() as ctx:
            const = ctx.enter_context(tc.tile_pool(name="const", bufs=1))
            wpool = ctx.enter_context(tc.tile_pool(name="wpool", bufs=1))
            xpool = ctx.enter_context(tc.tile_pool(name="xpool", bufs=1))
            qk = ctx.enter_context(tc.tile_pool(name="qk", bufs=1))
            vp = ctx.enter_context(tc.tile_pool(name="vp", bufs=1))
            epool = ctx.enter_context(tc.tile_pool(name="epool", bufs=1))
            apool = ctx.enter_context(tc.tile_pool(name="apool", bufs=1))
            opool = ctx.enter_context(tc.tile_pool(name="opool", bufs=1))
            rpool = ctx.enter_context(tc.tile_pool(name="rpool", bufs=1))

            # ---- constants / small DMAs ----
            bq_sb = [const.tile([128, 1], f32, tag=f"bq{m}", bufs=1,
                                name=f"bq_sb{m}") for m in range(2)]
            bk_sb = [const.tile([128, 1], f32, tag=f"bk{m}", bufs=1,
                                name=f"bk_sb{m}") for m in range(2)]
            bvb_sb = const.tile([128, DS], f32, tag="bvb", bufs=1, name="bvb_sb")
            em_sb = const.tile([128, KT], f32, tag="em", bufs=1, name="em_sb")
            ebias = const.tile([128, 1], f32, tag="ebias", bufs=1, name="ebias")

            # ---- big persistent SBUF ----
            wk_sb = wpool.tile([128, FT * 256], f16, tag="wk", bufs=1, name="wk")
            wq_sb = wpool.tile([128, FT * 256], f16, tag="wq", bufs=1, name="wq")
            wv_sb = wpool.tile([128, FT * VW], f16, tag="wv", bufs=1, name="wv")
            x01 = [xpool.tile([128, 1024], f16, tag=f"x01_{ft}", bufs=1,
                              name=f"x01_{ft}") for ft in range(FT)]
            x23 = [xpool.tile([128, 1024], f16, tag=f"x23_{ft}", bufs=1,
                              name=f"x23_{ft}") for ft in range(FT)]

            def xt(ft, nb):
                src = x01[ft] if nb < 2 else x23[ft]
                o = (nb % 2) * 512
                return src[:, o:o + 512]

            qt = [qk.tile([128, S], f16, tag=f"qt{m}", bufs=1, name=f"qt{m}")
                  for m in range(2)]
            kt_sb = [qk.tile([128, S], f16, tag=f"kt{m}", bufs=1, name=f"kt{m}")
                     for m in range(2)]
            vones = [vp.tile([128, VW], f16, tag=f"v{st}", bufs=1,
                             name=f"vones{st}") for st in range(KT)]
            # persistent per-side exp buffers, one iteration at a time
            esb = [epool.tile([128, KT * QBS], f16, tag=f"e{s}", bufs=1,
                              name=f"e{s}") for s in range(2)]
            # per-side ctx accumulators (f32): 4 qq x (64 ctx + 1 sum)
            acc = [apool.tile([128, 4 * (HD + 1)], f32, tag=f"acc{s}", bufs=1,
                              name=f"acc{s}") for s in range(2)]

            # ---- input DMAs in arrival-priority order ----
            for m in range(2):
                nc.sync.dma_start(bq_sb[m][:], bq_d.ap()[m])
                nc.sync.dma_start(bk_sb[m][:], bk_d.ap()[m])
            nc.sync.dma_start(em_sb[:], em_d.ap())
            nc.sync.dma_start(wk_sb[:], wkA_d.ap())
            nc.sync.dma_start(wq_sb[:], wqA_d.ap())
            for ft in range(FT):
                nc.sync.dma_start(x01[ft][:], xn01_d.ap()[ft])
            # wv/bvb/x23 are gated behind the first K-projection chain so
            # the critical path (wk, wq, x01) gets full DMA bandwidth.
            gated_dmas = []
            gated_dmas.append(nc.sync.dma_start(wv_sb[:], wvA_d.ap()))
            gated_dmas.append(nc.sync.dma_start(bvb_sb[:], bvb_d.ap()))
            for ft in range(FT):
                gated_dmas.append(
                    nc.sync.dma_start(x23[ft][:], xn23_d.ap()[ft]))

            nc.vector.memset(ebias[:], EXP_BIAS)
            # warm the ACT exp table while DMAs run
            warm = const.tile([128, 1], f32, tag="warm", bufs=1, name="warm")
            nc.scalar.activation(warm[:], ebias[:], AF.Exp)

            # ---- PSUM pools: scores A (4 banks) + B (2) + work (2x1) ----
            ps_a = ctx.enter_context(
                tc.tile_pool(name="ps_a", bufs=1, space="PSUM"))
            ps_b = ctx.enter_context(
                tc.tile_pool(name="ps_b", bufs=1, space="PSUM"))
            ps_w = ctx.enter_context(
                tc.tile_pool(name="ps_w", bufs=2, space="PSUM"))

            mm = nc.tensor.matmul

            # ---- work units ----
            def qk_proj_block(w_sb, bias_sb, dst, m, nb):
                ns = slice(nb * QBS, (nb + 1) * QBS)
                ps = ps_w.tile([128, QBS], f32, tag="w", name="pspj")
                last = None
                for ft in range(FT):
                    last = mm(
                       ps[:],
                       w_sb[:, ft * 256 + m * 128: ft * 256 + (m + 1) * 128],
                       xt(ft, nb),
                       start=(ft == 0), stop=(ft == FT - 1))
                nc.vector.tensor_scalar_add(dst[:, ns], ps[:], bias_sb[:])
                return last

            ghost_scr = const.tile([128, 1], f32, tag="gscr", bufs=1,
                                   name="ghost_scr")

            def ghost_block(nb):
                """PE ballast: a projection re-run whose result is unused.
                Keeps the tensor engine's HAM activity window busy during
                ACT-bound stretches so the PE clock stays at 2.4 GHz."""
                ps = ps_w.tile([128, QBS], f32, tag="w", name="ghost")
                for ft in range(FT):
                    mm(ps[:],
                       wq_sb[:, ft * 256: ft * 256 + 128],
                       xt(ft, nb),
                       start=(ft == 0), stop=(ft == FT - 1))
                nc.vector.tensor_copy(ghost_scr[:], ps[:, 0:1])

            def v_proj_block(st):
                nb, within = divmod(st, 4)
                ws = slice(within * 128, (within + 1) * 128)
                ps = ps_w.tile([128, QBS], f32, tag="w", name="pspjv")
                for ft in range(FT):
                    mm(ps[:, 0:VW],
                       xt(ft, nb)[:, ws],
                       wv_sb[:, ft * VW:(ft + 1) * VW],
                       start=(ft == 0), stop=(ft == FT - 1))
                nc.vector.tensor_scalar_mul(
                    vones[st][:], ps[:, 0:VW], em_sb[:, st:st + 1])
                for hh in range(HPC):
                    c = hh * (HD + 1) + HD
                    nc.vector.tensor_copy(
                        vones[st][:, c:c + 1], em_sb[:, st:st + 1])

            def scores_batch(it, side, bi):
                hp, qb = divmod(it, QB)
                k0, nk, pool = BATCH_PAT[side][bi]
                p0 = side * 64
                qs = slice(qb * QBS, (qb + 1) * QBS)
                pl = ps_a if pool == 0 else ps_b
                w = nk * QBS
                ps = pl.tile([128, 4 * QBS if pool == 0 else 2 * QBS], f32,
                             tag="sc", name=f"ps{'AB'[pool]}")
                for j in range(nk):
                    ktile = k0 + j
                    ks = slice(ktile * 128, (ktile + 1) * 128)
                    js = slice(j * QBS, (j + 1) * QBS)
                    mm(ps[:, js],
                       kt_sb[hp][p0:p0 + 64, ks], qt[hp][p0:p0 + 64, qs],
                       tile_position=(p0, 0))
                es = slice(k0 * QBS, k0 * QBS + w)
                nc.scalar.activation(esb[side][:, es], ps[:, 0:w],
                                     AF.Exp, bias=ebias[:], scale=0.125)

            def ctx_batch(it, side, bi):
                hp, _ = divmod(it, QB)
                hh = 2 * hp + side
                k0, nk, _ = BATCH_PAT[side][bi]
                e = esb[side]
                ps = ps_w.tile([128, QBS], f32, tag="w", name="pscx")
                for qq in range(4):
                    dst = ps[:, qq * (HD + 1):(qq + 1) * (HD + 1)]
                    for j in range(nk):
                        ktile = k0 + j
                        lo = ktile * QBS + qq * 128
                        mm(dst,
                           e[:, lo:lo + 128],
                           vones[ktile][:, hh * (HD + 1):(hh + 1) * (HD + 1)],
                           start=(j == 0), stop=(j == nk - 1))
                if bi == 0:
                    nc.vector.tensor_copy(acc[side][:], ps[:, 0:4 * (HD + 1)])
                else:
                    nc.vector.tensor_add(acc[side][:], acc[side][:],
                                         ps[:, 0:4 * (HD + 1)])

            def ctx_final(it, side, ots):
                hp, qb = divmod(it, QB)
                hh = 2 * hp + side
                for qq in range(4):
                    if side == 0:
                        ot = opool.tile([128, 128], f32, tag="ot", bufs=8,
                                        name="ot")
                        ots.append(ot)
                    else:
                        ot = ots[qq]
                    r = rpool.tile([128, 1], f32, tag="r", bufs=8, name="r")
                    a = acc[side]
                    nc.vector.reciprocal(
                        r[:], a[:, qq * (HD + 1) + HD: qq * (HD + 1) + HD + 1])
                    nc.vector.scalar_tensor_tensor(
                        ot[:, side * 64:(side + 1) * 64],
                        a[:, qq * (HD + 1): qq * (HD + 1) + HD], r[:],
                        bvb_sb[:, hh * HD:(hh + 1) * HD],
                        op0=OP.mult, op1=OP.add)
                    if side == 1:
                        qt_idx = qb * 4 + qq
                        nc.sync.dma_start(
                            out_d.ap()[qt_idx * 128:(qt_idx + 1) * 128,
                                       hp * 128:(hp + 1) * 128],
                            ot[:])

            # ---- emission schedule ----
            # Slot atoms per iteration: side A's 5 scores batches, ctx
            # lagging 3 batches; side A's trailing ctx batches spread into
            # side B's scores; side B's trailing ctx + final carry into
            # the next iteration's start (so every PE wait point has
            # independent work queued behind it and no gap grows past
            # ~1us — the HAM clock gate re-throttles on longer idles):
            #  0:(s,A,0) 1:(s,A,1) 2:(s,A,2) 3:(s,A,3) 4:(c,A,0) 5:(s,A,4)
            #  6:(c,A,1) 7:(s,B,0) 8:(c,A,2) 9:(s,B,1) 10:(c,A,3) 11:(s,B,2)
            #  12:(c,A,4) 13:(F,A) 14:(s,B,3) 15:(c,B,0) 16:(s,B,4) 17:(c,B,1)
            def K0(nb):
                return lambda: qk_proj_block(wk_sb, bk_sb[0], kt_sb[0], 0, nb)

            def K1(nb):
                return lambda: qk_proj_block(wk_sb, bk_sb[1], kt_sb[1], 1, nb)

            def Q0(nb):
                return lambda: qk_proj_block(wq_sb, bq_sb[0], qt[0], 0, nb)

            def Q1(nb):
                return lambda: qk_proj_block(wq_sb, bq_sb[1], qt[1], 1, nb)

            def V(st):
                return lambda: v_proj_block(st)

            # fillers[it] = list of (atom_idx, thunk): emit before that atom.
            # Placement honors DMA arrival order (wk, wq, x01, wv, x23) and
            # consumer deadlines (scores need kt/qt; ctx batch bi reads
            # vones[k0..k0+nk-1]; it N+1 reads carry ctx of it N).
            def G(nb):
                return lambda: ghost_block(nb)

            fillers = {
                0: [(1, K0(1)),
                    (2, V(0)), (2, V(1)), (2, V(2)), (2, V(3)),
                    (2, K1(0)), (2, Q1(0)), (2, K0(2)),
                    (3, V(4)), (3, V(5)),
                    (5, K0(3)),
                    (7, V(6)), (7, V(7)), (7, V(8)), (7, V(9)),
                    (9, V(10)), (9, V(11)),
                    (11, V(12)), (11, V(13)), (12, V(14)), (12, V(15)),
                    (17, Q0(1))],
                1: [(0, Q0(2)), (7, K1(1)), (14, G(0))],
                2: [(0, Q0(3)), (7, K1(2)), (14, G(1))],
                3: [(0, K1(3)), (7, Q1(1)), (14, G(2))],
                4: [(0, Q1(2)), (7, G(3)), (14, G(0))],
                5: [(0, Q1(3)), (7, G(1)), (14, G(2))],
                6: [(0, G(3)), (7, G(0)), (14, G(1))],
                7: [(0, G(2)), (7, G(3)), (14, G(0))],
            }

            # head: minimal work to unblock it0's first scores batch
            k_head = qk_proj_block(wk_sb, bk_sb[0], kt_sb[0], 0, 0)
            for dma in gated_dmas:
                add_dep_helper(dma.ins, k_head.ins, sync=True,
                               reason="dma-priority")
            qk_proj_block(wq_sb, bq_sb[0], qt[0], 0, 0)

            atoms = []
            for bi in range(NB_BATCH):
                atoms.append(("s", 0, bi))
                if bi >= 3:
                    atoms.append(("c", 0, bi - 3))
            for bi in range(NB_BATCH):
                atoms.append(("s", 1, bi))
                atoms.append(("c", 0, bi + 2) if bi <= 2 else ("c", 1, bi - 3))
                if bi == 2:
                    atoms.append(("F", 0, 0))

            carry = []  # side B trailing work, emitted at next it's start
            ots_map = {}
            for it in range(8):
                ots_map[it] = []
                flist = list(fillers.get(it, []))
                for c in carry[:2]:
                    c()
                cq = list(carry[2:])
                for idx, atom in enumerate(atoms):
                    while flist and flist[0][0] <= idx:
                        flist.pop(0)[1]()
                    kind, side, bi = atom
                    if kind == "s":
                        scores_batch(it, side, bi)
                    elif kind == "c":
                        ctx_batch(it, side, bi)
                    else:
                        ctx_final(it, 0, ots_map[it])
                    if cq:
                        cq.pop(0)()
                for f in flist:
                    f[1]()
                for c in cq:
                    c()
                carry = [
                    (lambda it=it: ctx_batch(it, 1, 2)),
                    (lambda it=it: ctx_batch(it, 1, 3)),
                    (lambda it=it: ctx_batch(it, 1, 4)),
                    (lambda it=it: ctx_final(it, 1, ots_map[it])),
                ]
            for c in carry:
                c()

    return nc


def _get_program(split_waits=True):
    key = ("nc", split_waits)
    if key not in _CACHE:
        _CACHE[key] = _build_program(split_waits)
    return _CACHE[key]


def _make_in_maps(hidden_states, attention_mask, Wq, bq, Wk, bk, Wv, bv):
    hidden = np.ascontiguousarray(np.asarray(hidden_states, dtype=np.float32))
    mask = np.asarray(attention_mask, dtype=np.float32)
    Wq = np.asarray(Wq, dtype=np.float32)
    Wk = np.asarray(Wk, dtype=np.float32)
    Wv = np.asarray(Wv, dtype=np.float32)
    bq = np.asarray(bq, dtype=np.float32)
    bk = np.asarray(bk, dtype=np.float32)
    bv = np.asarray(bv, dtype=np.float32)

    WqT = Wq.T  # [in, out]
    WkT = Wk.T
    WvT = Wv.T

    def pack_w(WT, cols):
        # [H, 256] -> [128, 8*256]: per f-tile 256-col blocks
        w = WT[:, cols].astype(np.float16)  # [1024, 256]
        return np.ascontiguousarray(
            w.reshape(FT, 128, 256).transpose(1, 0, 2).reshape(128, FT * 256))

    in_maps = []
    for c in range(NCORES):
        b, hg = divmod(c, HPC)
        cols = slice(hg * DS, (hg + 1) * DS)
        xT = hidden[b].T.astype(np.float16)  # [1024, 2048]
        xn01 = np.ascontiguousarray(
            xT[:, 0:1024].reshape(FT, 128, 1024))
        xn23 = np.ascontiguousarray(
            xT[:, 1024:2048].reshape(FT, 128, 1024))
        wkA = pack_w(WkT, cols)
        wqA = pack_w(WqT, cols)
        wv_base = WvT[:, cols].astype(np.float16)  # [1024, 256]
        wvA = np.zeros((128, FT * VW), np.float16)
        for ft in range(FT):
            blk = wv_base[ft * 128:(ft + 1) * 128]  # [128, 256]
            for hh in range(HPC):
                wvA[:, ft * VW + hh * (HD + 1): ft * VW + hh * (HD + 1) + HD] \
                    = blk[:, hh * HD:(hh + 1) * HD]
        bq_c = np.ascontiguousarray(bq[cols].reshape(2, 128, 1))
        bk_c = np.ascontiguousarray(bk[cols].reshape(2, 128, 1))
        bvb = np.ascontiguousarray(np.tile(bv[cols][None, :], (128, 1)))
        em = np.ascontiguousarray(
            np.exp(mask[b, 0, 0, :]).reshape(KT, 128).T.astype(np.float32))
        in_maps.append({
            "wkA": wkA, "wqA": wqA, "wvA": wvA,
            "xn01": xn01, "xn23": xn23,
            "bq": bq_c, "bk": bk_c, "bvb": bvb, "em": em,
        })
    return in_maps


def _assemble(results):
    out = np.empty((B, S, H), np.float32)
    for c in range(NCORES):
        b, hg = divmod(c, HPC)
        out[b][:, hg * DS:(hg + 1) * DS] = results[c]["out"]
    return out


def _run(in_maps, trace=False):
    from concourse.bass_utils import run_bass_kernel_spmd
    nc = _get_program()
    return run_bass_kernel_spmd(
        nc, in_maps, core_ids=list(range(NCORES)), trace=trace)


def kernel(**inputs):
    in_maps = _make_in_maps(**inputs)
    res = _run(in_maps, trace=False)
    return _assemble(res.results)
